# revision 1
# baseline (speedup 1.0000x reference)
"""Trainium2 Bass kernel for nn_Graph_to_Featuremaps_savemem.

Reference computation:
    scores[b,p,n] = s_res[b,p] + s_hid[b,n];  attn = softmax_n(scores)
    out[b,c,p]    = relu(sum_n attn[b,p,n] * (x[b,n,:] @ W)[c])

Key simplification: softmax over n is shift-invariant, so the per-pixel
s_res[b,p] term (the only use of res_feature / node_fea_for_res) cancels:
    attn[b,p,n] = softmax_n(s_hid[b,n])   (independent of p)
    out[b,c,p]  = relu(sum_n a[b,n] * nv[b,n,c])  broadcast over all pixels.

So the kernel is a tiny softmax-weighted matmul (per-batch (7,256)x(256,256))
followed by a 151 MB broadcast-write of the (B,C) result over H*W pixels.
Sharding: data-parallel over batch, 2 batches per core across 8 cores; the
small params (node_fea_for_hidden, weight) are replicated.

Hardware constraints shaping the structure:
- PE matmul / tensor-scalar / DMA-trigger instructions have a single
  sync-wait slot, so every PE operand pair must share one producer
  semaphore. All small inputs (w, x, nfh, identity, block-mask, ones) are
  packed host-side into ONE DRAM tensor loaded by ONE DMA; PSUM results are
  funneled through DVE copies.
- The kernel-tail drain also has limited wait slots, so the kernel keeps the
  total semaphore count low: only ACT (which triggers all DMAs), PE, DVE and
  the 8 HW DMA queues are used.
- matmul operands need base partition 0/32/64; x and the transpose identity
  live at rows 32:46 of the packed tile, everything else at base 0.
"""

import numpy as np

import concourse.bass as bass
import concourse.mybir as mybir
import concourse.tile as tile
from concourse.bass_utils import run_bass_kernel_spmd

B, NODES, HID, C, H, W = 16, 7, 256, 256, 96, 96
P = H * W                # 9216 pixels
NCORES = 8
BL = B // NCORES         # 2 local batches per core
BN = BL * NODES          # 14 (b,n) rows
WCHUNK = 9216            # broadcast tile width; P = 1 * WCHUNK
NCHUNK = P // WCHUNK

# Packed input layout: (128, CIN_COLS) float32
COL_W = 0        # cols 0:512, all rows: w[kh*128+k, c] at [k, kh*256+c]
COL_ID = 512     # cols 512:526, rows 32:46: identity(14)
COL_BM = 526     # cols 526:528, rows 0:14: block-diagonal mask (14, 2)
COL_XN = 528     # cols 528:784: row 0 = nfh; rows 32:46 = x[(b n), h]
COL_ONE = 784    # col 784, row 0: 1.0
CIN_COLS = 785
XROW = 32        # base partition for x / identity (must be 0, 32 or 64)

_cache: dict = {}


def _build_nc():
    nc = bass.Bass()
    dt = mybir.dt.float32
    cin_d = nc.declare_dram_parameter("cin", [128, CIN_COLS], dt, isOutput=False)
    out_d = nc.declare_dram_parameter("out", [BL, C, P], dt, isOutput=True)

    with tile.TileContext(nc) as tc:
        with (
            tc.tile_pool(name="sb", bufs=1) as sb,
            tc.tile_pool(name="ps", bufs=1, space=bass.MemorySpace.PSUM) as ps,
        ):
            cin = sb.tile([128, CIN_COLS], dt)
            nc.scalar.dma_start(out=cin[:], in_=cin_d[:])
            x_sl = cin[XROW : XROW + BN, COL_XN : COL_XN + HID]
            ident = cin[XROW : XROW + BN, COL_ID : COL_ID + BN]
            nfh_row = cin[0:1, COL_XN : COL_XN + HID]
            one_cin = cin[0:1, COL_ONE : COL_ONE + 1]

            ones11 = sb.tile([1, 1], dt)
            nc.vector.memset(ones11[:], 1.0)
            sb_w = sb.tile([128, 2 * C], dt)
            nc.vector.tensor_copy(out=sb_w[:], in_=cin[:, 0 : 2 * C])
            blkmask = sb.tile([BN, BL], dt)
            nc.vector.tensor_copy(out=blkmask[:], in_=cin[0:BN, COL_BM : COL_BM + BL])

            # PE-transpose x to (h, bn) layout, one (128, 14) tile per k-half.
            sbT = []
            for kh in range(2):
                p_t = ps.tile([128, BN], dt, tag=f"xT{kh}")
                nc.tensor.transpose(p_t[:], x_sl[:, kh * 128 : (kh + 1) * 128], ident)
                s_t = sb.tile([128, BN], dt, tag=f"sbT{kh}")
                nc.vector.tensor_copy(out=s_t[:], in_=p_t[:])
                sbT.append(s_t)
            # Transpose nfh row to a (128, kh) column pair via K=1 matmuls.
            p_nfh = ps.tile([128, 2], dt, tag="nfhT")
            for kh in range(2):
                nc.tensor.matmul(
                    p_nfh[:, kh : kh + 1],
                    nfh_row[:, kh * 128 : (kh + 1) * 128],
                    one_cin,
                    start=True,
                    stop=True,
                )
            sb_nfh_col = sb.tile([128, 2], dt)
            nc.vector.tensor_copy(out=sb_nfh_col[:], in_=p_nfh[:])

            # s_hid row (1, 14) and node_vals (14, 256), contracting h in 2 halves.
            ps_s = ps.tile([1, BN], dt, tag="s")
            ps_nv = ps.tile([BN, C], dt, tag="nv")
            for kh in range(2):
                nc.tensor.matmul(
                    ps_s[:],
                    sb_nfh_col[:, kh : kh + 1],
                    sbT[kh][:],
                    start=(kh == 0),
                    stop=(kh == 1),
                )
                nc.tensor.matmul(
                    ps_nv[:],
                    sbT[kh][:],
                    sb_w[:, kh * C : (kh + 1) * C],
                    start=(kh == 0),
                    stop=(kh == 1),
                )
            sb_nv = sb.tile([BN, C], dt)
            nc.vector.tensor_copy(out=sb_nv[:], in_=ps_nv[:])

            # Softmax over the 7 nodes (free dim), separately per local batch.
            e_row = sb.tile([1, BN], dt)
            denom = sb.tile([1, BL], dt)
            recip = sb.tile([1, BL], dt)
            a_row = sb.tile([1, BN], dt)
            for b in range(BL):
                nc.scalar.activation(
                    e_row[:, b * NODES : (b + 1) * NODES],
                    ps_s[:, b * NODES : (b + 1) * NODES],
                    mybir.ActivationFunctionType.Exp,
                    accum_out=denom[:, b : b + 1],
                )
            nc.vector.reciprocal(recip[:], denom[:])
            for b in range(BL):
                nc.vector.tensor_scalar_mul(
                    a_row[:, b * NODES : (b + 1) * NODES],
                    e_row[:, b * NODES : (b + 1) * NODES],
                    recip[:, b : b + 1],
                )

            # Transpose attn row to a column via K=1 matmul: ps_a[(b,n), 0] = a[b, n],
            # then expand into a block-diagonal (14, BL) matrix so one matmul per
            # c-half computes v for both local batches.
            ps_a = ps.tile([BN, 1], dt, tag="a")
            nc.tensor.matmul(ps_a[:], a_row[:], ones11[:], start=True, stop=True)
            sb_a = sb.tile([BN, 1], dt)
            nc.vector.tensor_copy(out=sb_a[:], in_=ps_a[:])
            rhs_a = sb.tile([BN, BL], dt)
            nc.vector.tensor_scalar_mul(rhs_a[:], blkmask[:], sb_a[:])

            # v[c, (ch, b)] = sum_n a[b, n] * nv[(b,n), c]; relu; broadcast; store.
            ps_v = ps.tile([128, 2 * BL], dt, tag="v")
            for ch in range(2):
                nc.tensor.matmul(
                    ps_v[:, ch * BL : (ch + 1) * BL],
                    sb_nv[:, ch * 128 : (ch + 1) * 128],
                    rhs_a[:],
                    start=True,
                    stop=True,
                )
            sb_v = sb.tile([128, 2 * BL], dt)
            nc.scalar.activation(sb_v[:], ps_v[:], mybir.ActivationFunctionType.Relu)
            # One broadcast tile + one DMA per local batch: out[b] is (256, P)
            # contiguous in DRAM, viewed as [p, ch, pix] with c = ch*128 + p.
            # Two DMAs let batch 1's broadcast fills overlap batch 0's store;
            # _fix_tail_drain spreads the resulting queue waits over spare
            # zero-wait tail drains.
            for b in range(BL):
                bc = sb.tile([128, 2, P], dt, tag=f"bc{b}")
                for ch in range(2):
                    j = ch * BL + b
                    nc.vector.tensor_copy(
                        out=bc[:, ch, :], in_=sb_v[:, j : j + 1].to_broadcast([128, P])
                    )
                nc.scalar.dma_start(
                    out=out_d[b].rearrange("(ch p) pix -> p ch pix", p=128),
                    in_=bc[:],
                )
    _fix_tail_drain(nc)
    return nc


def _fix_tail_drain(nc):
    """Walrus in this toolchain accepts very few sync waits per instruction, and
    Tile's kernel-tail drain waits on every semaphore. In this kernel the whole
    dataflow is one chain ending in the single output DMA: every other sem tick
    (input-DMA queue, PE, DVE, ACT) is strictly upstream of the output-DMA
    trigger, so waiting on the output queue's completion sem alone is
    sufficient. Strip the drain down to that one wait."""
    import bass_rust

    out_sem = None
    for ins in nc.inst_map.values():
        if type(ins).__name__ == "InstDMACopy" and "out_set" in str(ins):
            si = ins.sync_info
            if si is not None and len(si.on_update) > 0:
                out_sem = si.on_update[0].ant_name
    assert out_sem is not None, "output DMA completion sem not found"
    for ins in nc.inst_map.values():
        si = ins.sync_info
        if type(ins).__name__ == "InstDrain" and si is not None and len(si.on_wait) > 1:
            keep = [w for w in si.on_wait if w.ant_name == out_sem]
            assert len(keep) == 1, (out_sem, [w.ant_name for w in si.on_wait])
            ins.sync_info = bass_rust.SyncInfo(
                on_wait=keep, on_update=list(si.on_update)
            )


def _get_nc():
    if "nc" not in _cache:
        _cache["nc"] = _build_nc()
    return _cache["nc"]


def _pack_cin(x_shard, nfh, w):
    """Pack one core's inputs into the (128, CIN_COLS) tensor."""
    cin = np.zeros((128, CIN_COLS), dtype=np.float32)
    # w: [kh*128+k, c] -> [k, kh*256+c]
    cin[:, 0:C] = w[0:128, :]
    cin[:, C : 2 * C] = w[128:256, :]
    cin[XROW : XROW + BN, COL_ID : COL_ID + BN] = np.eye(BN, dtype=np.float32)
    for b in range(BL):
        cin[b * NODES : (b + 1) * NODES, COL_BM + b] = 1.0
    cin[0, COL_XN : COL_XN + HID] = nfh[:, 0]
    cin[XROW : XROW + BN, COL_XN : COL_XN + HID] = x_shard.reshape(BN, HID)
    cin[0, COL_ONE] = 1.0
    return cin


def _make_in_maps(input, node_fea_for_hidden, weight):
    x_full = np.asarray(input, dtype=np.float32)[0]  # (B, N, HID)
    nfh = np.asarray(node_fea_for_hidden, dtype=np.float32)
    w = np.asarray(weight, dtype=np.float32)
    return [
        {"cin": _pack_cin(x_full[i * BL : (i + 1) * BL], nfh, w)}
        for i in range(NCORES)
    ]


def _run(in_maps, trace=False, **kwargs):
    nc = _get_nc()
    return run_bass_kernel_spmd(nc, in_maps, list(range(NCORES)), trace=trace, **kwargs)


def kernel(input, res_feature, node_fea_for_res, node_fea_for_hidden, weight):
    in_maps = _make_in_maps(input, node_fea_for_hidden, weight)
    res = _run(in_maps)
    shards = [res.results[i]["out"] for i in range(NCORES)]  # each (BL, C, P)
    full = np.concatenate(shards, axis=0)  # (B, C, P)
    return full.reshape(B, C, H, W).astype(np.float32, copy=False)



# revision 11
# speedup vs baseline: 1.1179x; 1.1179x over previous
"""Trainium2 Bass kernel for nn_Graph_to_Featuremaps_savemem.

Reference computation:
    scores[b,p,n] = s_res[b,p] + s_hid[b,n];  attn = softmax_n(scores)
    out[b,c,p]    = relu(sum_n attn[b,p,n] * (x[b,n,:] @ W)[c])

Key simplification: softmax over n is shift-invariant, so the per-pixel
s_res[b,p] term (the only use of res_feature / node_fea_for_res) cancels:
    attn[b,p,n] = softmax_n(s_hid[b,n])   (independent of p)
    out[b,c,p]  = relu(sum_n a[b,n] * nv[b,n,c])  broadcast over all pixels.

Reassociating the tiny matmuls: with u = exp(s_hid) (unnormalized),
    xa[b,h] = sum_n u[b,n] x[b,n,h];  y[b,c] = xa[b] @ W;
    out[b,c,:] = relu(y[b,c] / sum_n u[b,n])  broadcast over 9216 pixels.

So the device work is a handful of tiny matmuls followed by a 151 MB
broadcast-write of the (B,C) result over H*W pixels. Sharding: data-parallel
over batch, 2 batches per core across 8 cores; params replicated.

The run is latency-bound around a single fixed resource: the ~424 GB/s
aggregate HBM write stream (one HW DMA queue fans packets over all 16 DMA
engines). Structure of this implementation:
- All transposes are done host-side at pack time (xT, nfh columns, x rows
  all pre-laid-out in one packed DRAM tensor) -> no PE transposes on the
  critical path.
- The output is cut into chunks with geometrically ramped widths: the first
  (128, 576) chunk is DVE-filled in ~0.3us and its DMA triggered
  immediately, then chunk sizes double while DVE fill (~950 GB/s) stays
  ahead of DMA drain (~424 GB/s) -> the write stream starts ~13us in and
  never starves.
- Normalization (1/sum u) and relu are folded into the broadcast fill
  itself: one DVE tensor_scalar per chunk computing
  max(y[c,j]*recip[b], 0) with a per-partition scalar.

Hardware constraints shaping the structure:
- PE matmul / tensor-scalar / DMA-trigger instructions have a single
  sync-wait slot, so every multi-operand instruction's operands are funneled
  through one producer engine: PE operand pairs come either both from the
  input DMA (same queue sem) or both from DVE (copies/memsets); the fill's
  two operands (y columns, recip broadcast) are both DVE copies of PSUM.
- All DMAs (input + output chunks) are triggered by the ACT engine onto one
  HW queue, so the kernel-tail drain only needs that queue's completion
  semaphore (_fix_tail_drain strips the rest).
"""

import numpy as np

import concourse.bass as bass
import concourse.mybir as mybir
import concourse.tile as tile
from concourse.bass_utils import run_bass_kernel_spmd

B, NODES, HID, C, H, W = 16, 7, 256, 256, 96, 96
P = H * W                # 9216 pixels
NCORES = 8
BL = B // NCORES         # 2 local batches per core
BN = BL * NODES          # 14 (b,n) rows

# Packed input layout: (128, NCOLS) float32
COL_W = 0        # cols 0:512: w[kh*128+k, c] at [k, kh*256+c]
COL_XT = 512     # cols 512:540: xT[kh*128+k, bn] at [k, 512+kh*14+bn]
COL_NFH = 540    # cols 540:542: nfh[kh*128+k, 0] at [k, 540+kh]
COL_XR = 542     # cols 542:798, rows 0:14: x[(b n), h]
COL_BM = 798     # cols 798:800, rows 0:14: block-diagonal mask (14, 2)
NCOLS = 800

# Output chunk schedule per (local batch, c-half): pixel-range widths.
# First chunks are small so the first DMA triggers early; widths ramp so the
# ~950 GB/s DVE fill stays ahead of the ~424 GB/s DMA drain. Exactly 7
# output chunks + 1 input DMA = 8, one per Tile HW-queue semaphore: a 9th
# DMA would wrap onto a reused sem and need a second (unsupported) sync
# wait on the trigger instruction.
RAMP = [576, 1536, 3072, 4032]
assert sum(RAMP) == P

_cache: dict = {}


def _build_nc():
    nc = bass.Bass()
    dt = mybir.dt.float32
    fp = mybir.ActivationFunctionType
    alu = mybir.AluOpType
    cin_d = nc.declare_dram_parameter("cin", [128, NCOLS], dt, isOutput=False)
    out_d = nc.declare_dram_parameter("out", [BL, C, P], dt, isOutput=True)

    with tile.TileContext(nc) as tc:
        with (
            tc.tile_pool(name="sb", bufs=1) as sb,
            tc.tile_pool(name="ps", bufs=1, space=bass.MemorySpace.PSUM) as ps,
        ):
            # Constants via DVE memset: no DMA dependency, runs during load.
            ones14 = sb.tile([BN, 1], dt)
            nc.vector.memset(ones14[:], 1.0)
            onesr = sb.tile([1, 128], dt)
            nc.vector.memset(onesr[:], 1.0)

            cin = sb.tile([128, NCOLS], dt)
            nc.scalar.dma_start(out=cin[:], in_=cin_d[:])

            # s_hid column: s[(b,n)] = sum_h xT[h,(b,n)] * nfh[h], 2 k-halves.
            ps_s = ps.tile([BN, 1], dt, tag="s")
            for kh in range(2):
                nc.tensor.matmul(
                    ps_s[:],
                    cin[:, COL_XT + kh * BN : COL_XT + (kh + 1) * BN],
                    cin[:, COL_NFH + kh : COL_NFH + kh + 1],
                    start=(kh == 0),
                    stop=(kh == 1),
                )

            # DVE funnel copies (overlap the PE work above).
            x_rows = sb.tile([BN, HID], dt)
            nc.vector.tensor_copy(out=x_rows[:], in_=cin[0:BN, COL_XR : COL_XR + HID])
            blkmask = sb.tile([BN, BL], dt)
            nc.vector.tensor_copy(out=blkmask[:], in_=cin[0:BN, COL_BM : COL_BM + BL])
            w_sb = sb.tile([128, 2 * C], dt)
            nc.vector.tensor_copy(out=w_sb[:], in_=cin[:, COL_W : COL_W + 2 * C])

            # u = exp(s) (unnormalized attention), expanded into a
            # block-diagonal (14, BL) matrix via the packed mask.
            sb_u = sb.tile([BN, 1], dt)
            nc.scalar.activation(sb_u[:], ps_s[:], fp.Exp)
            # Funnel u through DVE so the tensor_scalar below has a single
            # producer engine (one sync-wait slot on tensor_scalar).
            sb_u2 = sb.tile([BN, 1], dt)
            nc.vector.tensor_copy(out=sb_u2[:], in_=sb_u[:])
            ublk = sb.tile([BN, BL], dt)
            nc.vector.tensor_scalar_mul(ublk[:], blkmask[:], sb_u2[:])

            # xa[h, b] = sum_n x[(b,n), h] * u[b, n], per k-half; and the
            # softmax denominator row denom[b] = sum_n u[b, n].
            ps_xa = [
                ps.tile([128, BL], dt, tag=f"xa{kh}", name=f"ps_xa{kh}")
                for kh in range(2)
            ]
            for kh in range(2):
                nc.tensor.matmul(
                    ps_xa[kh][:],
                    x_rows[:, kh * 128 : (kh + 1) * 128],
                    ublk[:],
                    start=True,
                    stop=True,
                )
            ps_den = ps.tile([1, BL], dt, tag="den")
            nc.tensor.matmul(ps_den[:], ones14[:], ublk[:], start=True, stop=True)

            sb_xa = []
            for kh in range(2):
                t = sb.tile([128, BL], dt, tag=f"sxa{kh}", name=f"sb_xa{kh}")
                nc.vector.tensor_copy(out=t[:], in_=ps_xa[kh][:])
                sb_xa.append(t)
            recip = sb.tile([1, BL], dt)
            nc.vector.reciprocal(recip[:], ps_den[:])

            # y[c, (ch,b)] = sum_h W[h, c] * xa[h, b], c split into halves on
            # partitions, h accumulated over k-halves.
            ps_y = ps.tile([128, 2 * BL], dt, tag="y")
            for ch in range(2):
                for kh in range(2):
                    nc.tensor.matmul(
                        ps_y[:, ch * BL : (ch + 1) * BL],
                        w_sb[:, kh * C + ch * 128 : kh * C + ch * 128 + 128],
                        sb_xa[kh][:],
                        start=(kh == 0),
                        stop=(kh == 1),
                    )
            # recip broadcast to all 128 partitions via K=1 matmul (runs
            # after yT on PE; its DVE input was ready earlier).
            ps_rbc = ps.tile([128, BL], dt, tag="rbc")
            nc.tensor.matmul(ps_rbc[:], onesr[:], recip[:], start=True, stop=True)

            sb_y = sb.tile([128, 2 * BL], dt)
            nc.vector.tensor_copy(out=sb_y[:], in_=ps_y[:])
            sb_rbc = sb.tile([128, BL], dt)
            nc.vector.tensor_copy(out=sb_rbc[:], in_=ps_rbc[:])

            # Broadcast fills + chunked stores. Each fill is one DVE
            # tensor_scalar: max(y[c, j] * recip[b], 0) broadcast over the
            # chunk's pixels; each store is a plain 2D DMA of 128 c-lines.
            segs = []
            lo = 0
            for w_seg in RAMP:
                segs.append((0, 0, lo, w_seg))
                lo += w_seg
            segs += [(0, 1, 0, P), (1, 0, 0, P), (1, 1, 0, P)]

            bc = {}
            for b in range(BL):
                for ch in range(2):
                    bc[(b, ch)] = sb.tile(
                        [128, P], dt, tag=f"bc{b}{ch}", name=f"bc{b}{ch}"
                    )
            for b, ch, lo, w_seg in segs:
                j = ch * BL + b
                t = bc[(b, ch)]
                nc.vector.tensor_scalar(
                    out=t[:, lo : lo + w_seg],
                    in0=sb_y[:, j : j + 1].to_broadcast([128, w_seg]),
                    scalar1=sb_rbc[:, b : b + 1],
                    scalar2=0.0,
                    op0=alu.mult,
                    op1=alu.max,
                )
                nc.scalar.dma_start(
                    out=out_d[b][ch * 128 : (ch + 1) * 128, lo : lo + w_seg],
                    in_=t[:, lo : lo + w_seg],
                )
    _fix_tail_drain(nc)
    return nc


def _fix_tail_drain(nc):
    """Walrus in this toolchain accepts very few sync waits per instruction,
    and Tile's kernel-tail drain waits on every semaphore. Every instruction
    here is strictly upstream of the last output-chunk DMA, and all DMAs
    share the single ACT HW queue, so waiting on that queue's completion sem
    alone (at its final count) is sufficient. Strip the drain to that wait."""
    import bass_rust

    out_sem = None
    for ins in nc.inst_map.values():
        if type(ins).__name__ == "InstDMACopy" and "out_set" in str(ins):
            si = ins.sync_info
            if si is not None and len(si.on_update) > 0:
                out_sem = si.on_update[0].ant_name
    assert out_sem is not None, "output DMA completion sem not found"
    for ins in nc.inst_map.values():
        si = ins.sync_info
        if type(ins).__name__ == "InstDrain" and si is not None and len(si.on_wait) > 1:
            keep = [w for w in si.on_wait if w.ant_name == out_sem]
            assert len(keep) == 1, (out_sem, [w.ant_name for w in si.on_wait])
            ins.sync_info = bass_rust.SyncInfo(
                on_wait=keep, on_update=list(si.on_update)
            )


def _get_nc():
    if "nc" not in _cache:
        _cache["nc"] = _build_nc()
    return _cache["nc"]


def _pack_cin(x_shard, nfh, w):
    """Pack one core's inputs into the (128, NCOLS) tensor."""
    cin = np.zeros((128, NCOLS), dtype=np.float32)
    cin[:, COL_W : COL_W + C] = w[0:128, :]
    cin[:, COL_W + C : COL_W + 2 * C] = w[128:256, :]
    xr = x_shard.reshape(BN, HID)
    xT = np.ascontiguousarray(xr.T)  # (256, 14)
    cin[:, COL_XT : COL_XT + BN] = xT[0:128]
    cin[:, COL_XT + BN : COL_XT + 2 * BN] = xT[128:256]
    cin[:, COL_NFH] = nfh[0:128, 0]
    cin[:, COL_NFH + 1] = nfh[128:256, 0]
    cin[0:BN, COL_XR : COL_XR + HID] = xr
    for b in range(BL):
        cin[b * NODES : (b + 1) * NODES, COL_BM + b] = 1.0
    return cin


def _make_in_maps(input, node_fea_for_hidden, weight):
    x_full = np.asarray(input, dtype=np.float32)[0]  # (B, N, HID)
    nfh = np.asarray(node_fea_for_hidden, dtype=np.float32)
    w = np.asarray(weight, dtype=np.float32)
    return [
        {"cin": _pack_cin(x_full[i * BL : (i + 1) * BL], nfh, w)}
        for i in range(NCORES)
    ]


def _run(in_maps, trace=False, **kwargs):
    nc = _get_nc()
    return run_bass_kernel_spmd(nc, in_maps, list(range(NCORES)), trace=trace, **kwargs)


def kernel(input, res_feature, node_fea_for_res, node_fea_for_hidden, weight):
    in_maps = _make_in_maps(input, node_fea_for_hidden, weight)
    res = _run(in_maps)
    shards = [res.results[i]["out"] for i in range(NCORES)]  # each (BL, C, P)
    full = np.concatenate(shards, axis=0)  # (B, C, P)
    return full.reshape(B, C, H, W).astype(np.float32, copy=False)


# revision 16
# speedup vs baseline: 1.1420x; 1.0216x over previous
"""Trainium2 Bass kernel for nn_Graph_to_Featuremaps_savemem.

Reference computation:
    scores[b,p,n] = s_res[b,p] + s_hid[b,n];  attn = softmax_n(scores)
    out[b,c,p]    = relu(sum_n attn[b,p,n] * (x[b,n,:] @ W)[c])

Key simplification: softmax over n is shift-invariant, so the per-pixel
s_res[b,p] term (the only use of res_feature / node_fea_for_res) cancels:
    attn[b,p,n] = softmax_n(s_hid[b,n])   (independent of p)
    out[b,c,p]  = relu(sum_n a[b,n] * nv[b,n,c])  broadcast over all pixels.

Reassociating the tiny matmuls: with u = exp(s_hid) (unnormalized),
    xa[b,h] = sum_n u[b,n] x[b,n,h];  y[b,c] = xa[b] @ W;
    out[b,c,:] = relu(y[b,c] / sum_n u[b,n])  broadcast over 9216 pixels.

So the device work is a handful of tiny matmuls followed by a 151 MB
broadcast-write of the (B,C) result over H*W pixels. Sharding: data-parallel
over batch, 2 batches per core across 8 cores; params replicated.

The run is latency-bound around a single fixed resource: the ~424 GB/s
aggregate HBM write stream (one HW DMA queue fans packets over all 16 DMA
engines). Structure of this implementation:
- All transposes are done host-side at pack time (xT, nfh columns, x rows
  all pre-laid-out in one packed DRAM tensor) -> no PE transposes on the
  critical path.
- The output is cut into chunks with geometrically ramped widths: the first
  (128, 576) chunk is DVE-filled in ~0.3us and its DMA triggered
  immediately, then chunk sizes double while DVE fill (~950 GB/s) stays
  ahead of DMA drain (~424 GB/s) -> the write stream starts ~13us in and
  never starves.
- Normalization (1/sum u) and relu are folded into the broadcast fill
  itself: one DVE tensor_scalar per chunk computing
  max(y[c,j]*recip[b], 0) with a per-partition scalar.

Hardware constraints shaping the structure:
- PE matmul / tensor-scalar / DMA-trigger instructions have a single
  sync-wait slot, so every multi-operand instruction's operands are funneled
  through one producer engine: PE operand pairs come either both from the
  input DMA (same queue sem) or both from DVE (copies/memsets); the fill's
  two operands (y columns, recip broadcast) are both DVE copies of PSUM.
- All DMAs (input + output chunks) are triggered by the ACT engine onto one
  HW queue, so the kernel-tail drain only needs that queue's completion
  semaphore (_fix_tail_drain strips the rest).
"""

import numpy as np

import concourse.bass as bass
import concourse.mybir as mybir
import concourse.tile as tile
from concourse.bass_utils import run_bass_kernel_spmd

B, NODES, HID, C, H, W = 16, 7, 256, 256, 96, 96
P = H * W                # 9216 pixels
NCORES = 8
BL = B // NCORES         # 2 local batches per core
BN = BL * NODES          # 14 (b,n) rows

# Packed inputs, two DRAM tensors loaded by two DMAs on one FIFO queue:
# cin0 (128, 32), small so it lands early and starts the PE chain ~1us
# sooner than the bulky weight block behind it.
C0_XT = 0        # cols 0:28: xT[kh*128+k, bn] at [k, kh*14+bn]
C0_NFH = 28      # cols 28:30: nfh[kh*128+k, 0] at [k, 28+kh]
C0_BM = 30       # cols 30:32, rows 0:14: block-diagonal mask (14, 2)
NCOLS0 = 32
# cin1 (128, 768): w k-halves + x rows.
C1_W = 0         # cols 0:512: w[kh*128+k, c] at [k, kh*256+c]
C1_XR = 512      # cols 512:768, rows 0:14: x[(b n), h]
NCOLS1 = 768

# Output chunk schedule per (local batch, c-half): pixel-range widths.
# First chunks are small so the first DMA triggers early; widths ramp so the
# ~950 GB/s DVE fill stays ahead of the ~424 GB/s DMA drain. Exactly 7
# output chunks + 1 input DMA = 8, one per Tile HW-queue semaphore: a 9th
# DMA would wrap onto a reused sem and need a second (unsupported) sync
# wait on the trigger instruction.
RAMP = [576, 1536, 3072, 4032]
assert sum(RAMP) == P

_cache: dict = {}


def _build_nc():
    nc = bass.Bass()
    dt = mybir.dt.float32
    fp = mybir.ActivationFunctionType
    alu = mybir.AluOpType
    cin0_d = nc.declare_dram_parameter("cin0", [128, NCOLS0], dt, isOutput=False)
    cin1_d = nc.declare_dram_parameter("cin1", [128, NCOLS1], dt, isOutput=False)
    out_d = nc.declare_dram_parameter("out", [BL, C, P], dt, isOutput=True)

    with tile.TileContext(nc) as tc:
        with (
            tc.tile_pool(name="sb", bufs=1) as sb,
            tc.tile_pool(name="ps", bufs=1, space=bass.MemorySpace.PSUM) as ps,
        ):
            # Constants via DVE memset: no DMA dependency, runs during load.
            ones14 = sb.tile([BN, 1], dt)
            nc.vector.memset(ones14[:], 1.0)
            onesr = sb.tile([1, 128], dt)
            nc.vector.memset(onesr[:], 1.0)

            cin0 = sb.tile([128, NCOLS0], dt)
            nc.scalar.dma_start(out=cin0[:], in_=cin0_d[:])
            cin1 = sb.tile([128, NCOLS1], dt)
            nc.scalar.dma_start(out=cin1[:], in_=cin1_d[:])

            # s_hid column: s[(b,n)] = sum_h xT[h,(b,n)] * nfh[h], 2 k-halves.
            ps_s = ps.tile([BN, 1], dt, tag="s")
            for kh in range(2):
                nc.tensor.matmul(
                    ps_s[:],
                    cin0[:, C0_XT + kh * BN : C0_XT + (kh + 1) * BN],
                    cin0[:, C0_NFH + kh : C0_NFH + kh + 1],
                    start=(kh == 0),
                    stop=(kh == 1),
                )

            # DVE funnel copies (overlap the PE work above).
            blkmask = sb.tile([BN, BL], dt)
            nc.vector.tensor_copy(out=blkmask[:], in_=cin0[0:BN, C0_BM : C0_BM + BL])
            x_rows = sb.tile([BN, HID], dt)
            nc.vector.tensor_copy(out=x_rows[:], in_=cin1[0:BN, C1_XR : C1_XR + HID])
            w_sb = sb.tile([128, 2 * C], dt)
            nc.vector.tensor_copy(out=w_sb[:], in_=cin1[:, C1_W : C1_W + 2 * C])

            # u = exp(s) (unnormalized attention), expanded into a
            # block-diagonal (14, BL) matrix via the packed mask.
            sb_u = sb.tile([BN, 1], dt)
            nc.scalar.activation(sb_u[:], ps_s[:], fp.Exp)
            # Funnel u through DVE so the tensor_scalar below has a single
            # producer engine (one sync-wait slot on tensor_scalar).
            sb_u2 = sb.tile([BN, 1], dt)
            nc.vector.tensor_copy(out=sb_u2[:], in_=sb_u[:])
            ublk = sb.tile([BN, BL], dt)
            nc.vector.tensor_scalar_mul(ublk[:], blkmask[:], sb_u2[:])

            # Softmax denominator first (it feeds the recip -> recip-broadcast
            # chain that must be ready before the first fill), then xa.
            ps_den = ps.tile([1, BL], dt, tag="den")
            nc.tensor.matmul(ps_den[:], ones14[:], ublk[:], start=True, stop=True)
            # xa[h, b] = sum_n x[(b,n), h] * u[b, n], per k-half.
            ps_xa = [
                ps.tile([128, BL], dt, tag=f"xa{kh}", name=f"ps_xa{kh}")
                for kh in range(2)
            ]
            for kh in range(2):
                nc.tensor.matmul(
                    ps_xa[kh][:],
                    x_rows[:, kh * 128 : (kh + 1) * 128],
                    ublk[:],
                    start=True,
                    stop=True,
                )

            recip = sb.tile([1, BL], dt)
            nc.vector.reciprocal(recip[:], ps_den[:])
            sb_xa = []
            for kh in range(2):
                t = sb.tile([128, BL], dt, tag=f"sxa{kh}", name=f"sb_xa{kh}")
                nc.vector.tensor_copy(out=t[:], in_=ps_xa[kh][:])
                sb_xa.append(t)

            # recip broadcast to all 128 partitions via K=1 matmul; scheduled
            # before yT so the PE never stalls waiting on it afterwards.
            ps_rbc = ps.tile([128, BL], dt, tag="rbc")
            nc.tensor.matmul(ps_rbc[:], onesr[:], recip[:], start=True, stop=True)
            # y[c, (ch,b)] = sum_h W[h, c] * xa[h, b], c split into halves on
            # partitions, h accumulated over k-halves.
            ps_y = ps.tile([128, 2 * BL], dt, tag="y")
            for ch in range(2):
                for kh in range(2):
                    nc.tensor.matmul(
                        ps_y[:, ch * BL : (ch + 1) * BL],
                        w_sb[:, kh * C + ch * 128 : kh * C + ch * 128 + 128],
                        sb_xa[kh][:],
                        start=(kh == 0),
                        stop=(kh == 1),
                    )

            sb_rbc = sb.tile([128, BL], dt)
            nc.vector.tensor_copy(out=sb_rbc[:], in_=ps_rbc[:])
            sb_y = sb.tile([128, 2 * BL], dt)
            nc.vector.tensor_copy(out=sb_y[:], in_=ps_y[:])

            # Broadcast fills + chunked stores. Each fill is one DVE
            # tensor_scalar: max(y[c, j] * recip[b], 0) broadcast over the
            # chunk's pixels; each store is a plain 2D DMA of 128 c-lines.
            segs = []
            lo = 0
            for w_seg in RAMP:
                segs.append((0, 0, lo, w_seg))
                lo += w_seg
            segs += [(0, 1, 0, P), (1, 0, 0, P), (1, 1, 0, P)]

            bc = {}
            for b in range(BL):
                for ch in range(2):
                    bc[(b, ch)] = sb.tile(
                        [128, P], dt, tag=f"bc{b}{ch}", name=f"bc{b}{ch}"
                    )
            for b, ch, lo, w_seg in segs:
                j = ch * BL + b
                t = bc[(b, ch)]
                nc.vector.tensor_scalar(
                    out=t[:, lo : lo + w_seg],
                    in0=sb_y[:, j : j + 1].to_broadcast([128, w_seg]),
                    scalar1=sb_rbc[:, b : b + 1],
                    scalar2=0.0,
                    op0=alu.mult,
                    op1=alu.max,
                )
                nc.scalar.dma_start(
                    out=out_d[b][ch * 128 : (ch + 1) * 128, lo : lo + w_seg],
                    in_=t[:, lo : lo + w_seg],
                )
    _fix_tail_drain(nc)
    return nc


def _fix_tail_drain(nc):
    """Walrus in this toolchain accepts very few sync waits per instruction.
    Two post-passes, both relying on the fact that all 9 DMAs here run
    through the ACT engine's single FIFO HW queue (completion order = issue
    order), and every instruction is strictly upstream of the last
    output-chunk DMA:

    1. Tile rotates DMA completions over 8 DMAHW sems; the 9th dma_start
       reuses the first input DMA's sem and its trigger gains a second
       sync-wait (queue-slot reuse guard) on top of its DVE fill wait. That
       guard is implied by the fill (the fill is transitively downstream of
       the input DMA), so drop the DMAHW wait and keep the DVE wait.
    2. Tile's kernel-tail drain waits on every semaphore; waiting on the
       final output chunk's completion sem alone is sufficient."""
    import bass_rust

    out_sem = None
    for ins in nc.inst_map.values():
        si = ins.sync_info
        if type(ins).__name__ != "InstDMACopy" or si is None:
            continue
        if len(si.on_wait) > 1:
            keep = [w for w in si.on_wait if not w.ant_name.startswith("DMAHW")]
            assert len(keep) == 1, [w.ant_name for w in si.on_wait]
            ins.sync_info = bass_rust.SyncInfo(
                on_wait=keep, on_update=list(si.on_update)
            )
        if "out_set" in str(ins) and len(si.on_update) > 0:
            out_sem = si.on_update[0].ant_name
    assert out_sem is not None, "output DMA completion sem not found"
    for ins in nc.inst_map.values():
        si = ins.sync_info
        if type(ins).__name__ == "InstDrain" and si is not None and len(si.on_wait) > 1:
            keep = [w for w in si.on_wait if w.ant_name == out_sem]
            assert len(keep) == 1, (out_sem, [w.ant_name for w in si.on_wait])
            ins.sync_info = bass_rust.SyncInfo(
                on_wait=keep, on_update=list(si.on_update)
            )


def _get_nc():
    if "nc" not in _cache:
        _cache["nc"] = _build_nc()
    return _cache["nc"]


def _pack_cin(x_shard, nfh, w):
    """Pack one core's inputs into the two packed tensors."""
    cin0 = np.zeros((128, NCOLS0), dtype=np.float32)
    xr = x_shard.reshape(BN, HID)
    xT = np.ascontiguousarray(xr.T)  # (256, 14)
    cin0[:, C0_XT : C0_XT + BN] = xT[0:128]
    cin0[:, C0_XT + BN : C0_XT + 2 * BN] = xT[128:256]
    cin0[:, C0_NFH] = nfh[0:128, 0]
    cin0[:, C0_NFH + 1] = nfh[128:256, 0]
    for b in range(BL):
        cin0[b * NODES : (b + 1) * NODES, C0_BM + b] = 1.0
    cin1 = np.zeros((128, NCOLS1), dtype=np.float32)
    cin1[:, C1_W : C1_W + C] = w[0:128, :]
    cin1[:, C1_W + C : C1_W + 2 * C] = w[128:256, :]
    cin1[0:BN, C1_XR : C1_XR + HID] = xr
    return cin0, cin1


def _make_in_maps(input, node_fea_for_hidden, weight):
    x_full = np.asarray(input, dtype=np.float32)[0]  # (B, N, HID)
    nfh = np.asarray(node_fea_for_hidden, dtype=np.float32)
    w = np.asarray(weight, dtype=np.float32)
    maps = []
    for i in range(NCORES):
        cin0, cin1 = _pack_cin(x_full[i * BL : (i + 1) * BL], nfh, w)
        maps.append({"cin0": cin0, "cin1": cin1})
    return maps


def _run(in_maps, trace=False, **kwargs):
    nc = _get_nc()
    return run_bass_kernel_spmd(nc, in_maps, list(range(NCORES)), trace=trace, **kwargs)


def kernel(input, res_feature, node_fea_for_res, node_fea_for_hidden, weight):
    in_maps = _make_in_maps(input, node_fea_for_hidden, weight)
    res = _run(in_maps)
    shards = [res.results[i]["out"] for i in range(NCORES)]  # each (BL, C, P)
    full = np.concatenate(shards, axis=0)  # (B, C, P)
    return full.reshape(B, C, H, W).astype(np.float32, copy=False)
